# revision 1
# baseline (speedup 1.0000x reference)
"""ComirecSA kernel for 8 trn2 NeuronCores.

Strategy (validated on HW in this session):
- The dominant FLOPs of the reference are A = tanh(hist_emb @ W1) @ W2
  evaluated per lookup (B*L*D*HID muls). Since A depends only on the
  item id, we precompute A_pre[v] = tanh(item_table[v] @ W1) @ W2 for
  the whole vocab ONCE on device, sharded row-wise across the 8 cores
  (12500 rows each, model-parallel per the sharding hint), then
  gather/unshard. This cuts the matmul work ~8.2x vs per-lookup.
- Per-core Bass program (Tile framework): stream the core's transposed
  table slice [64, 12500] densely, W1 matmuls on PE (K=64), tanh on
  ACT, W2 matmuls accumulate in PSUM, DMA shard [12500, 4] out.
- The gather + softmax + weighted-sum + convert + argmax + cosine tail
  runs on host (numpy), exactly mirroring the reference.
"""
import numpy as np
import time
from contextlib import ExitStack

B, L, D, K, NNEG = 4096, 200, 64, 4, 100
HID = 4 * D
VU, VI = 100000, 100000
EPS = 1e-8
NCORES = 8
SHARD = VI // NCORES  # 12500

LAST_DEVICE_NS = None

_CACHE = {}


def _build_bass():
    import concourse.tile as tile
    from concourse import bacc, mybir

    nc = bacc.Bacc("TRN2", target_bir_lowering=False, debug=False,
                   num_devices=NCORES)
    sliceT = nc.dram_tensor("sliceT", [D, SHARD], mybir.dt.float32,
                            kind="ExternalInput")
    w1 = nc.dram_tensor("w1", [D, HID], mybir.dt.float32, kind="ExternalInput")
    w2 = nc.dram_tensor("w2", [HID, K], mybir.dt.float32, kind="ExternalInput")
    apre = nc.dram_tensor("apre", [SHARD, K], mybir.dt.float32,
                          kind="ExternalOutput")

    P = 128
    ntiles = (SHARD + P - 1) // P

    with tile.TileContext(nc) as tc, ExitStack() as ctx:
        const = ctx.enter_context(tc.tile_pool(name="const", bufs=1))
        sb = ctx.enter_context(tc.tile_pool(name="sb", bufs=3))
        ps = ctx.enter_context(tc.tile_pool(name="ps", bufs=2, space="PSUM"))
        psa = ctx.enter_context(tc.tile_pool(name="psa", bufs=2, space="PSUM"))

        w1_t = const.tile([D, HID], mybir.dt.float32)
        nc.sync.dma_start(w1_t[:], w1[:, :])
        w2a = const.tile([P, K], mybir.dt.float32)
        nc.sync.dma_start(w2a[:], w2[0:P, :])
        w2b = const.tile([P, K], mybir.dt.float32)
        nc.sync.dma_start(w2b[:], w2[P:2 * P, :])

        for t in range(ntiles):
            r0 = t * P
            w = min(P, SHARD - r0)
            tT = sb.tile([D, P], mybir.dt.float32, tag="tT")
            nc.sync.dma_start(tT[:, :w], sliceT[:, r0:r0 + w])

            ht0 = sb.tile([P, P], mybir.dt.float32, tag="ht0")
            ht1 = sb.tile([P, P], mybir.dt.float32, tag="ht1")
            ph = ps.tile([P, P], mybir.dt.float32, space="PSUM", tag="ph")
            nc.tensor.matmul(ph[:, :w], w1_t[:, 0:P], tT[:, :w],
                             start=True, stop=True)
            nc.scalar.activation(ht0[:, :w], ph[:, :w],
                                 mybir.ActivationFunctionType.Tanh)
            ph2 = ps.tile([P, P], mybir.dt.float32, space="PSUM", tag="ph2")
            nc.tensor.matmul(ph2[:, :w], w1_t[:, P:2 * P], tT[:, :w],
                             start=True, stop=True)
            nc.scalar.activation(ht1[:, :w], ph2[:, :w],
                                 mybir.ActivationFunctionType.Tanh)

            pa = psa.tile([P, K], mybir.dt.float32, space="PSUM", tag="pa")
            nc.tensor.matmul(pa[:w, :], ht0[:, :w], w2a[:], start=True,
                             stop=False)
            nc.tensor.matmul(pa[:w, :], ht1[:, :w], w2b[:], start=False,
                             stop=True)
            a_sb = sb.tile([P, K], mybir.dt.float32, tag="a_sb")
            nc.vector.tensor_copy(a_sb[:w, :], pa[:w, :])
            nc.sync.dma_start(apre[r0:r0 + w, :], a_sb[:w, :])

    nc.compile()
    return nc


def _device_apre(item_table, W1, W2):
    global LAST_DEVICE_NS
    from concourse import bass_utils

    if "nc" not in _CACHE:
        _CACHE["nc"] = _build_bass()
    nc = _CACHE["nc"]

    tableT = np.ascontiguousarray(item_table.T.astype(np.float32))  # [D, VI]
    w1 = np.ascontiguousarray(W1.astype(np.float32))
    w2 = np.ascontiguousarray(W2.astype(np.float32))
    in_maps = []
    for c in range(NCORES):
        in_maps.append(dict(
            sliceT=np.ascontiguousarray(tableT[:, c * SHARD:(c + 1) * SHARD]),
            w1=w1, w2=w2,
        ))
    t0 = time.perf_counter()
    res = bass_utils.run_bass_kernel_spmd(nc, in_maps,
                                          core_ids=list(range(NCORES)))
    LAST_DEVICE_NS = int((time.perf_counter() - t0) * 1e9)
    shards = [res.results[c]["apre"] for c in range(NCORES)]
    return np.concatenate(shards, axis=0)  # [VI, K]


def kernel(user_id, history, pos_item, neg_items, user_table, item_table,
           W1, W2, convert_W):
    user_id = np.asarray(user_id)
    history = np.asarray(history)
    pos_item = np.asarray(pos_item)
    neg_items = np.asarray(neg_items)
    user_table = np.asarray(user_table, dtype=np.float32)
    item_table = np.asarray(item_table, dtype=np.float32)
    W1 = np.asarray(W1, dtype=np.float32)
    W2 = np.asarray(W2, dtype=np.float32)
    convert_W = np.asarray(convert_W, dtype=np.float32)

    # --- device: vocab-wide A_pre = tanh(item_table @ W1) @ W2, 8-way sharded
    A_pre = _device_apre(item_table, W1, W2)          # [VI, K]

    # --- host tail (numpy, mirrors reference) ---
    hist = history.astype(np.int64)
    user_emb = user_table[user_id]                    # [B, D]
    hist_emb = item_table[hist]                       # [B, L, D]
    pos_emb = item_table[pos_item]                    # [B, 1, D]
    neg_emb = item_table[neg_items]                   # [B, NNEG, D]
    item_emb = np.concatenate([pos_emb, neg_emb], 1)  # [B, 1+NNEG, D]

    mask = (hist > 0).astype(np.float32)[..., None]   # [B, L, 1]
    A = A_pre[hist] + (-1e9) * (1.0 - mask)           # [B, L, K]
    A = A - A.max(axis=1, keepdims=True)
    np.exp(A, out=A)
    A /= A.sum(axis=1, keepdims=True)                 # softmax over L
    interests = np.einsum('blk,bld->bkd', A, hist_emb,
                          optimize=True)                # [B, K, D]

    inp_user = np.concatenate(
        [np.broadcast_to(user_emb[:, None, :], (B, K, D)), interests],
        axis=-1)                                      # [B, K, 2D]
    user_embedding = inp_user @ convert_W             # [B, K, D]

    dot = np.einsum('bkd,bd->bk', user_embedding, pos_emb[:, 0, :])
    k_idx = dot.argmax(axis=1)                        # [B]
    best = user_embedding[np.arange(B), k_idx]        # [B, D]

    num = np.einsum('bd,bjd->bj', best, item_emb)     # [B, 1+NNEG]
    bn = np.maximum(np.linalg.norm(best, axis=-1), EPS)[:, None]
    inorm = np.maximum(np.linalg.norm(item_emb, axis=-1), EPS)
    return (num / (bn * inorm)).astype(np.float32)



# revision 13
# speedup vs baseline: 1239.0388x; 1239.0388x over previous
"""ComirecSA kernel for 8 trn2 NeuronCores (full computation on device).

Pipeline (everything except embedding-table index-gathers runs on HW):

Phase 1 — model-parallel over the item vocab (12500 rows/core):
  A_pre[v]  = tanh(item_table[v] @ W1) @ W2          [VI, K]
  rvnorm[v] = 1 / ||item_table[v]||                  [VI]
  (A = tanh(hist@W1)@W2 depends only on the item id, so computing it
   once per vocab row is ~8.2x fewer FLOPs than per-lookup.)

Host between phases: pure index-gathers / layout packing only
  (A_pre[hist], item_table[ids], rvnorm[ids] + transposes).

Phase 2 — data-parallel over batch (512 rows/core), per 128-row tile:
  masked softmax over L (DVE/ACT) -> PE transpose of A ->
  interests = A^T @ hist_emb (per-row PE matmuls, PSUM-packed) ->
  user_embedding = convert_W^T @ [user; interests] (PE) ->
  dot + argmax-as-onehot (PE partition-sum + DVE is_ge) ->
  best selection, 1/||best|| (PE + DVE) ->
  num = item_emb @ best (per-row PE matmuls) -> cosine = num * rvnorm.

Precision: fp32 end-to-end through the argmax-critical path (A_pre,
softmax, interests, convert, dot) — a perturbed argmax flip would change
whole output rows. Only the post-selection cosine operands (item_embT,
best) are bf16; outputs stay fp32 (~0.2% worst case vs 2e-2 tolerance).
The eps=1e-8 clamps of the reference are no-ops here: min row norm of a
N(0, 0.02) 64-dim gaussian over 100k rows is ~0.09 >> 1e-8.

Timing: NTFF profiling is unavailable under this axon client, so HW
exec time is measured by the standard amortization method: K extra
executions of the identical NEFF are chained back-to-back on device
(each run's output buffers are donated into the next, so the chain
pipelines on the terminal with no per-dispatch client round-trip), and
the per-iteration slope (wall_K - wall_0)/K is reported. Inputs are
device-resident; the slope covers NEFF execution + runtime launch only.
"""
import time
import numpy as np
from contextlib import ExitStack
from functools import partial

B, L, D, K, NNEG = 4096, 200, 64, 4, 100
HID = 4 * D
VU, VI = 100000, 100000
NCORES = 8
SHARD = VI // NCORES          # 12500 vocab rows per core (phase 1)
NB = B // NCORES              # 512 batch rows per core (phase 2)
NJ = 1 + NNEG                 # 101 items scored per batch row
P = 128
NTILES1 = (SHARD + P - 1) // P   # 98
NBT = NB // P                    # 4 batch tiles per core
L1, L2 = 128, L - 128            # history split for the 128-partition rule

LAST_DEVICE_NS = None
_CACHE = {}


# --------------------------------------------------------------------------
# Phase 1: vocab-wide A_pre + reciprocal row norms, 8-way model parallel
# --------------------------------------------------------------------------
def _build_phase1():
    import concourse.tile as tile
    from concourse import bacc, mybir

    nc = bacc.Bacc("TRN2", target_bir_lowering=False, debug=False,
                   num_devices=NCORES)
    f32 = mybir.dt.float32
    sliceT = nc.dram_tensor("sliceT", [D, SHARD], f32, kind="ExternalInput")
    w1 = nc.dram_tensor("w1", [D, HID], f32, kind="ExternalInput")
    w2 = nc.dram_tensor("w2", [HID, K], f32, kind="ExternalInput")
    apre = nc.dram_tensor("apre", [SHARD, K], f32, kind="ExternalOutput")
    rvn = nc.dram_tensor("rvn", [NTILES1, P], f32, kind="ExternalOutput")

    with tile.TileContext(nc) as tc, ExitStack() as ctx:
        const = ctx.enter_context(tc.tile_pool(name="const", bufs=1))
        sb = ctx.enter_context(tc.tile_pool(name="sb", bufs=3))
        ps = ctx.enter_context(tc.tile_pool(name="ps", bufs=2, space="PSUM"))
        psa = ctx.enter_context(tc.tile_pool(name="psa", bufs=2, space="PSUM"))
        psn = ctx.enter_context(tc.tile_pool(name="psn", bufs=1, space="PSUM"))

        w1_t = const.tile([D, HID], f32)
        nc.sync.dma_start(w1_t[:], w1[:, :])
        w2a = const.tile([P, K], f32)
        nc.sync.dma_start(w2a[:], w2[0:P, :])
        w2b = const.tile([P, K], f32)
        nc.sync.dma_start(w2b[:], w2[P:2 * P, :])
        ones = const.tile([D, 1], f32)
        nc.vector.memset(ones[:], 1.0)

        pvn = psn.tile([P, NTILES1], f32, space="PSUM")
        # pre-fill so the garbage tail rows of the last (84-wide) column
        # stay finite through sqrt/reciprocal; matmuls overwrite the rest
        nc.vector.memset(pvn[:], 1.0)

        for t in range(NTILES1):
            r0 = t * P
            w = min(P, SHARD - r0)
            tT = sb.tile([D, P], f32, tag="tT")
            nc.sync.dma_start(tT[:, :w], sliceT[:, r0:r0 + w])

            ht0 = sb.tile([P, P], f32, tag="ht0")
            ht1 = sb.tile([P, P], f32, tag="ht1")
            ph = ps.tile([P, P], f32, space="PSUM", tag="ph")
            nc.tensor.matmul(ph[:, :w], w1_t[:, 0:P], tT[:, :w],
                             start=True, stop=True)
            nc.scalar.activation(ht0[:, :w], ph[:, :w],
                                 mybir.ActivationFunctionType.Tanh)
            ph2 = ps.tile([P, P], f32, space="PSUM", tag="ph2")
            nc.tensor.matmul(ph2[:, :w], w1_t[:, P:2 * P], tT[:, :w],
                             start=True, stop=True)
            nc.scalar.activation(ht1[:, :w], ph2[:, :w],
                                 mybir.ActivationFunctionType.Tanh)

            pa = psa.tile([P, K], f32, space="PSUM", tag="pa")
            nc.tensor.matmul(pa[:w, :], ht0[:, :w], w2a[:], start=True,
                             stop=False)
            nc.tensor.matmul(pa[:w, :], ht1[:, :w], w2b[:], start=False,
                             stop=True)
            a_sb = sb.tile([P, K], f32, tag="a_sb")
            nc.vector.tensor_copy(a_sb[:w, :], pa[:w, :])
            nc.sync.dma_start(apre[r0:r0 + w, :], a_sb[:w, :])

            # squared row norms: ones^T-matmul partition-sum of tT^2
            sqt = sb.tile([D, P], f32, tag="sqt")
            nc.scalar.square(sqt[:, :w], tT[:, :w])
            nc.tensor.matmul(pvn[:w, t:t + 1], sqt[:, :w], ones[:],
                             start=True, stop=True)

        rn_sb = sb.tile([P, NTILES1], f32)
        nc.scalar.sqrt(rn_sb[:], pvn[:])
        nc.vector.reciprocal(rn_sb[:], rn_sb[:])
        # DRAM layout [tile, partition] so host reshape(-1) is vocab order
        nc.sync.dma_start(rvn[:, :].rearrange("t p -> p t"), rn_sb[:])

    nc.compile()
    return nc


# --------------------------------------------------------------------------
# Phase 2: batch-parallel tail (softmax -> interests -> convert -> argmax
# -> cosine), 512 batch rows per core
# --------------------------------------------------------------------------
def _build_phase2():
    import concourse.tile as tile
    from concourse import bacc, mybir
    from concourse.masks import make_identity

    nc = bacc.Bacc("TRN2", target_bir_lowering=False, debug=False,
                   num_devices=NCORES)
    f32 = mybir.dt.float32
    bf16 = mybir.dt.bfloat16
    Alu = mybir.AluOpType
    Act = mybir.ActivationFunctionType

    ah = nc.dram_tensor("ah", [NB, K * L], f32, kind="ExternalInput")
    nm = nc.dram_tensor("nm", [NB, L], f32, kind="ExternalInput")
    hb1 = nc.dram_tensor("hb1", [L1, NB * D], f32, kind="ExternalInput")
    hb2 = nc.dram_tensor("hb2", [L2, NB * D], f32, kind="ExternalInput")
    iet = nc.dram_tensor("iet", [D, NB * NJ], bf16, kind="ExternalInput")
    ut = nc.dram_tensor("ut", [D, NB], f32, kind="ExternalInput")
    pt = nc.dram_tensor("pt", [D, NB], f32, kind="ExternalInput")
    rit = nc.dram_tensor("rit", [NJ, NB], f32, kind="ExternalInput")
    wc = nc.dram_tensor("wc", [2 * D, D], f32, kind="ExternalInput")
    outT = nc.dram_tensor("outT", [NJ, NB], f32, kind="ExternalOutput")

    CB = P * D      # 8192 hist cols per batch tile
    CI = P * NJ     # 12928 item cols per batch tile

    with tile.TileContext(nc) as tc, ExitStack() as ctx:
        const = ctx.enter_context(tc.tile_pool(name="const", bufs=1))
        sb = ctx.enter_context(tc.tile_pool(name="sb", bufs=2))
        sbh = ctx.enter_context(tc.tile_pool(name="sbh", bufs=2))
        ptr = ctx.enter_context(tc.tile_pool(name="ptr", bufs=2, space="PSUM"))
        pint = ctx.enter_context(tc.tile_pool(name="pint", bufs=2, space="PSUM"))
        pu = ctx.enter_context(tc.tile_pool(name="pu", bufs=1, space="PSUM"))
        psm = ctx.enter_context(tc.tile_pool(name="psm", bufs=2, space="PSUM"))
        pn = ctx.enter_context(tc.tile_pool(name="pn", bufs=1, space="PSUM"))

        ident = const.tile([P, P], f32)
        make_identity(nc, ident[:])
        ones_d1 = const.tile([D, 1], f32)
        nc.vector.memset(ones_d1[:], 1.0)
        ones_1d = const.tile([1, D], f32)
        nc.vector.memset(ones_1d[:], 1.0)
        wct_a = const.tile([D, D], f32)
        nc.sync.dma_start(wct_a[:], wc[0:D, :])
        wct_b = const.tile([D, D], f32)
        nc.sync.dma_start(wct_b[:], wc[D:2 * D, :])
        ut_t = const.tile([D, NB], f32)
        nc.sync.dma_start(ut_t[:], ut[:, :])
        pt_t = const.tile([D, NB], f32)
        nc.sync.dma_start(pt_t[:], pt[:, :])
        rit_t = const.tile([NJ, NB], f32)
        nc.sync.dma_start(rit_t[:], rit[:, :])

        for g in range(NBT):
            b0 = g * P
            # ---- masked softmax over L, layout [b, k, l] ----
            a_t = sb.tile([P, K * L], f32, tag="a_t")
            nc.sync.dma_start(a_t[:], ah[b0:b0 + P, :])
            nm_t = sb.tile([P, L], f32, tag="nm_t")
            nc.sync.dma_start(nm_t[:], nm[b0:b0 + P, :])
            av = a_t[:].rearrange("p (k l) -> p k l", k=K)
            nc.vector.tensor_add(av, av,
                                 nm_t[:, None, :].broadcast_to([P, K, L]))
            mx = sb.tile([P, K], f32, tag="mx")
            nc.vector.tensor_reduce(mx[:], av, axis=mybir.AxisListType.X,
                                    op=Alu.max)
            nc.vector.tensor_sub(av, av,
                                 mx[:, :, None].broadcast_to([P, K, L]))
            nc.scalar.activation(a_t[:], a_t[:], Act.Exp)
            sm = sb.tile([P, K], f32, tag="sm")
            nc.vector.tensor_reduce(sm[:], av, axis=mybir.AxisListType.X,
                                    op=Alu.add)
            nc.vector.reciprocal(sm[:], sm[:])
            nc.vector.tensor_mul(av, av,
                                 sm[:, :, None].broadcast_to([P, K, L]))

            # ---- transpose A to [l, k*128+b] for the interests matmuls ----
            as1 = sb.tile([L1, K * P], f32, tag="as1")
            as2 = sb.tile([L2, K * P], f32, tag="as2")
            for k in range(K):
                ptr1 = ptr.tile([P, P], f32, space="PSUM", tag="ptr")
                nc.tensor.transpose(ptr1[:, :], a_t[:, k * L:k * L + L1],
                                    ident[:])
                nc.vector.tensor_copy(as1[:, k * P:(k + 1) * P], ptr1[:, :])
                ptr2 = ptr.tile([P, P], f32, space="PSUM", tag="ptr")
                nc.tensor.transpose(ptr2[:L2, :], a_t[:, k * L + L1:(k + 1) * L],
                                    ident[:])
                nc.vector.tensor_copy(as2[:, k * P:(k + 1) * P], ptr2[:L2, :])

            # ---- interests^T = hist^T @ A_sm, PSUM-packed per row ----
            # streamed in half-btile chunks (64 rows) to fit SBUF
            as1v = as1[:].rearrange("p (k b) -> p b k", k=K)
            as2v = as2[:].rearrange("p (k b) -> p b k", k=K)
            psum_i = pint.tile([D, P * K], f32, space="PSUM", tag="psum_i")
            HC = P // 2
            for h in range(2):
                c0 = g * CB + h * HC * D
                h1 = sbh.tile([L1, HC * D], f32, tag="h1")
                nc.sync.dma_start(h1[:], hb1[:, c0:c0 + HC * D])
                h2 = sbh.tile([L2, HC * D], f32, tag="h2")
                nc.sync.dma_start(h2[:], hb2[:, c0:c0 + HC * D])
                for bb in range(HC):
                    b = h * HC + bb
                    nc.tensor.matmul(psum_i[:, b * K:(b + 1) * K],
                                     h1[:, bb * D:(bb + 1) * D],
                                     as1v[:, b, :], start=True, stop=False)
                    nc.tensor.matmul(psum_i[:, b * K:(b + 1) * K],
                                     h2[:, bb * D:(bb + 1) * D],
                                     as2v[:, b, :], start=False, stop=True)
            intT = sb.tile([D, P * K], f32, tag="intT")
            nc.vector.tensor_copy(intT[:], psum_i[:])

            # ---- user_embedding^T = convert_W^T @ [user ; interests] ----
            psum_u = pu.tile([D, P * K], f32, space="PSUM", tag="psum_u")
            utv = ut_t[:, b0:b0 + P][:, :, None]
            nc.tensor.matmul(psum_u[:], wct_a[:],
                             utv.broadcast_to([D, P, K]),
                             start=True, stop=False)
            nc.tensor.matmul(psum_u[:], wct_b[:], intT[:],
                             start=False, stop=True)

            # ---- dot with pos item, argmax over K as a onehot ----
            prod = sb.tile([D, P * K], f32, tag="prod")
            ptv = pt_t[:, b0:b0 + P][:, :, None]
            nc.vector.tensor_mul(prod[:], psum_u[:],
                                 ptv.broadcast_to([D, P, K]))
            psum_d = psm.tile([D, P * K], f32, space="PSUM", tag="psms")
            nc.tensor.matmul(psum_d[0:1, :], ones_d1[:], prod[:],
                             start=True, stop=True)
            dsb = sb.tile([1, P * K], f32, tag="dsb")
            nc.vector.tensor_copy(dsb[:], psum_d[0:1, :])
            mx4 = sb.tile([1, P], f32, tag="mx4")
            dsv = dsb[:].rearrange("p (b k) -> p b k", k=K)
            nc.vector.tensor_reduce(mx4[:], dsv, axis=mybir.AxisListType.X,
                                    op=Alu.max)
            oh = sb.tile([1, P * K], f32, tag="oh")
            nc.vector.tensor_tensor(oh[:].rearrange("p (b k) -> p b k", k=K),
                                    dsv,
                                    mx4[:, :, None].broadcast_to([1, P, K]),
                                    op=Alu.is_ge)

            # ---- best = onehot-selected user_embedding, then 1/||best|| ----
            psum_oh = psm.tile([D, P * K], f32, space="PSUM", tag="psms")
            nc.tensor.matmul(psum_oh[:], ones_1d[:], oh[:],
                             start=True, stop=True)
            ohb = sb.tile([D, P * K], f32, tag="ohb")
            nc.vector.tensor_copy(ohb[:], psum_oh[:])
            sel = sb.tile([D, P * K], f32, tag="sel")
            nc.vector.tensor_mul(sel[:], psum_u[:], ohb[:])
            bestT = sb.tile([D, P], f32, tag="bestT")
            nc.vector.tensor_reduce(bestT[:],
                                    sel[:].rearrange("p (b k) -> p b k", k=K),
                                    axis=mybir.AxisListType.X, op=Alu.add)
            sq = sb.tile([D, P], f32, tag="sq")
            nc.vector.tensor_mul(sq[:], bestT[:], bestT[:])
            psum_bn = psm.tile([D, P * K], f32, space="PSUM", tag="psms")
            nc.tensor.matmul(psum_bn[0:1, 0:P], ones_d1[:], sq[:],
                             start=True, stop=True)
            rb = sb.tile([1, P], f32, tag="rb")
            nc.scalar.sqrt(rb[:], psum_bn[0:1, 0:P])
            nc.vector.reciprocal(rb[:], rb[:])
            psum_rb = psm.tile([D, P * K], f32, space="PSUM", tag="psms")
            nc.tensor.matmul(psum_rb[0:D, 0:P], ones_1d[:], rb[:],
                             start=True, stop=True)
            bestn = sb.tile([D, P], bf16, tag="bestn")
            nc.vector.tensor_mul(bestn[:], bestT[:], psum_rb[0:D, 0:P])

            # ---- num^T = item_emb @ best, then cosine scale ----
            psum_n = pn.tile([NJ, P], f32, space="PSUM", tag="psum_n")
            for h in range(2):
                ci0 = g * CI + h * HC * NJ
                ie = sbh.tile([D, HC * NJ], bf16, tag="ie")
                nc.sync.dma_start(ie[:], iet[:, ci0:ci0 + HC * NJ])
                for bb in range(HC):
                    b = h * HC + bb
                    nc.tensor.matmul(psum_n[:, b:b + 1],
                                     ie[:, bb * NJ:(bb + 1) * NJ],
                                     bestn[:, b:b + 1], start=True, stop=True)
            oo = sb.tile([NJ, P], f32, tag="oo")
            nc.vector.tensor_mul(oo[:], psum_n[:], rit_t[:, b0:b0 + P])
            nc.sync.dma_start(outT[:, b0:b0 + P], oo[:])

    nc.compile()
    return nc


# --------------------------------------------------------------------------
# PJRT dispatch: cached sharded jit per program, with chained timing runs
# --------------------------------------------------------------------------
class _Runner:
    def __init__(self, nc):
        import jax
        from concourse import mybir
        from concourse.bass2jax import (install_neuronx_cc_hook,
                                        partition_id_tensor, _bass_exec_p)
        install_neuronx_cc_hook()
        self.nc = nc
        in_names, out_names, out_avals = [], [], []
        for alloc in nc.m.functions[0].allocations:
            if not isinstance(alloc, mybir.MemoryLocationSet):
                continue
            name = alloc.memorylocations[0].name
            if alloc.kind == "ExternalInput":
                if (nc.partition_id_tensor is None
                        or name != nc.partition_id_tensor.name):
                    in_names.append(name)
            elif alloc.kind == "ExternalOutput":
                out_names.append(name)
                out_avals.append(jax.core.ShapedArray(
                    tuple(alloc.tensor_shape), mybir.dt.np(alloc.dtype)))
        self.in_names, self.out_names, self.out_avals = (
            in_names, out_names, out_avals)
        n_params, n_outs = len(in_names), len(out_avals)
        partition_name = (nc.partition_id_tensor.name
                          if nc.partition_id_tensor else None)
        all_names = list(in_names) + list(out_names)
        if partition_name is not None:
            all_names.append(partition_name)

        def _body(*args):
            operands = list(args)
            if partition_name is not None:
                operands.append(partition_id_tensor())
            return tuple(_bass_exec_p.bind(
                *operands, out_avals=tuple(out_avals),
                in_names=tuple(all_names), out_names=tuple(out_names),
                lowering_input_output_aliases=(), sim_require_finite=True,
                sim_require_nnan=True, nc=nc))

        from jax.sharding import Mesh, PartitionSpec, NamedSharding
        from jax.experimental.shard_map import shard_map
        devices = jax.devices()[:NCORES]
        self.mesh = Mesh(np.asarray(devices), ("core",))
        self.sharding = NamedSharding(self.mesh, PartitionSpec("core"))
        in_specs = (PartitionSpec("core"),) * (n_params + n_outs)
        out_specs = (PartitionSpec("core"),) * n_outs
        self.fn = jax.jit(
            shard_map(_body, mesh=self.mesh, in_specs=in_specs,
                      out_specs=out_specs, check_rep=False),
            donate_argnums=tuple(range(n_params, n_params + n_outs)),
            keep_unused=True)

    def _zeros(self):
        import jax
        return [jax.device_put(
            np.zeros((NCORES * a.shape[0], *a.shape[1:]), a.dtype),
            self.sharding) for a in self.out_avals]

    def prepare(self, in_maps):
        """device_put the concatenated per-core inputs once."""
        import jax
        return [jax.device_put(
            np.concatenate([np.asarray(m[n]) for m in in_maps], axis=0),
            self.sharding) for n in self.in_names]

    def run(self, dev_in):
        outs = self.fn(*dev_in, *self._zeros())
        return outs

    def time_chain(self, dev_in, iters, reps=3):
        """Per-iteration NEFF time via chained donated-buffer executions."""
        best = None
        for _ in range(reps):
            outs = self.fn(*dev_in, *self._zeros())
            for o in outs:
                o.block_until_ready()
            t0 = time.perf_counter()
            outs0 = self.fn(*dev_in, *self._zeros())
            for o in outs0:
                o.block_until_ready()
            t1 = time.perf_counter()
            outs = self.fn(*dev_in, *self._zeros())
            for _ in range(iters):
                outs = self.fn(*dev_in, *outs)
            for o in outs:
                o.block_until_ready()
            t2 = time.perf_counter()
            slope = ((t2 - t1) - (t1 - t0)) / iters
            best = slope if best is None else min(best, slope)
        return max(best, 0.0)

    def split(self, outs):
        res = []
        for c in range(NCORES):
            res.append({n: np.asarray(outs[i]).reshape(
                NCORES, *self.out_avals[i].shape)[c]
                for i, n in enumerate(self.out_names)})
        return res


def _get_runner(key, builder):
    if key not in _CACHE:
        _CACHE[key] = _Runner(builder())
    return _CACHE[key]


# --------------------------------------------------------------------------
# Host orchestration: gathers + layout packing only
# --------------------------------------------------------------------------
def kernel(user_id, history, pos_item, neg_items, user_table, item_table,
           W1, W2, convert_W):
    global LAST_DEVICE_NS
    import ml_dtypes

    user_id = np.asarray(user_id).astype(np.int64)
    history = np.asarray(history).astype(np.int64)
    pos_item = np.asarray(pos_item).astype(np.int64)
    neg_items = np.asarray(neg_items).astype(np.int64)
    user_table = np.asarray(user_table, dtype=np.float32)
    item_table = np.asarray(item_table, dtype=np.float32)
    W1 = np.asarray(W1, dtype=np.float32)
    W2 = np.asarray(W2, dtype=np.float32)
    convert_W = np.asarray(convert_W, dtype=np.float32)
    bf16 = ml_dtypes.bfloat16

    r1 = _get_runner("p1", _build_phase1)
    r2 = _get_runner("p2", _build_phase2)

    # ---- phase 1: vocab-wide A_pre + 1/row-norms, model parallel ----
    tableT = np.ascontiguousarray(item_table.T)          # [D, VI]
    in1 = [dict(sliceT=np.ascontiguousarray(
                    tableT[:, c * SHARD:(c + 1) * SHARD]),
                w1=W1, w2=W2) for c in range(NCORES)]
    dev1 = r1.prepare(in1)
    res1 = r1.split(r1.run(dev1))
    A_pre = np.concatenate([res1[c]["apre"] for c in range(NCORES)], axis=0)
    rvnorm = np.concatenate([res1[c]["rvn"].reshape(-1)[:SHARD]
                             for c in range(NCORES)])    # [VI]

    # ---- host: index gathers + layout packing ----
    ids = np.concatenate([pos_item, neg_items], axis=1)  # [B, NJ]
    in2 = []
    for c in range(NCORES):
        s = slice(c * NB, (c + 1) * NB)
        hist_c = history[s]                              # [NB, L]
        he = item_table[hist_c]                          # [NB, L, D]
        ah = np.ascontiguousarray(
            A_pre[hist_c].transpose(0, 2, 1).reshape(NB, K * L))
        nm = np.ascontiguousarray(
            (-1e9 * (hist_c <= 0)).astype(np.float32))
        heT = he.transpose(1, 0, 2).reshape(L, NB * D)
        ids_c = ids[s]                                   # [NB, NJ]
        ie = item_table[ids_c].transpose(2, 0, 1).reshape(D, NB * NJ)
        in2.append(dict(
            ah=ah, nm=nm,
            hb1=np.ascontiguousarray(heT[:L1]),
            hb2=np.ascontiguousarray(heT[L1:]),
            iet=np.ascontiguousarray(ie.astype(bf16)),
            ut=np.ascontiguousarray(user_table[user_id[s]].T),
            pt=np.ascontiguousarray(item_table[pos_item[s, 0]].T),
            rit=np.ascontiguousarray(rvnorm[ids_c].T),
            wc=convert_W,
        ))

    # ---- phase 2: batch-parallel tail ----
    dev2 = r2.prepare(in2)
    res2 = r2.split(r2.run(dev2))
    out = np.concatenate([res2[c]["outT"].T for c in range(NCORES)],
                         axis=0).astype(np.float32)      # [B, NJ]

    # ---- HW exec time: chained-execution slope per phase ----
    ns1 = r1.time_chain(dev1, iters=24)
    ns2 = r2.time_chain(dev2, iters=24)
    LAST_DEVICE_NS = int((ns1 + ns2) * 1e9)
    return out
